# revision 1
# baseline (speedup 1.0000x reference)
"""Correlation cost-volume kernel for Trainium2 (Bass/Tile).

Problem: in1, in2: [B=8, C=128, H=96, W=128] fp32.
Output: [B, 81, H, W] where out[b, dy*9+dx, y, x] =
    mean_c( in1[b,c,y,x] * in2_pad[b,c,y+dy,x+dx] ),
with in2 zero-padded by 4 in both spatial dims (max_displacement=4).

Strategy (data-parallel over batch, one sample per NeuronCore):
  - For each in1 row y, compute the Gram band against the 9 surrounding
    (padded) in2 rows with TensorE matmuls: stationary = in1[:, y, :]
    ([C=128, W=128]), moving = padded in2 rows y..y+8 ([C=128, 3x136] per
    matmul, 3 matmuls) -> PSUM G[x, (dy, x')] where
    G = sum_c in1[c,y,x] * in2p[c, y+dy, x'].
  - Copy PSUM->SBUF in 32-partition groups, keeping only the 40-wide
    window W[x, dy, u] = G[x, dy, 32*(x//32)+u] each pixel group needs
    (pure access patterns only: mixed partition+byte strides in DMA APs
    miscompute on HW - the DGE wraps the per-partition byte carry).
  - Extract the banded taps with 32 partition-strided SBUF->SBUF DMAs
    (s = x mod 32): t2[x, dy*9+dx] = W[x, dy, s+dx].
  - PE-transpose the [128 x, 81 k] band tile to [81, 128] and DMA
    straight into the output cost volume rows, scaling by 1/C on the
    way.

Matmuls run in float32r (full PE rate for N>=256, ~7e-4 scale-relative
error vs fp64 reference -- measured on hardware).
"""

import numpy as np

import concourse.bass as bass
import concourse.mybir as mybir
from concourse import bacc
from concourse.bass_utils import run_bass_kernel_spmd
from concourse.masks import make_identity
from concourse.tile import TileContext

B = 8
C = 128
H = 96
W = 128
D = 9  # 2*max_disp + 1
K = D * D  # 81 output channels
PAD = 4
WP = W + 2 * PAD  # 136
FP32 = mybir.dt.float32
FP32R = mybir.dt.float32r

N_CORES = 8


def build_bass(h: int = H):
    """Build the per-core Bass program for a [C, h, W] sample."""
    hp = h + 2 * PAD
    nc = bacc.Bacc(None, target_bir_lowering=False)
    in1 = nc.dram_tensor("in1", [C, h, W], FP32R, kind="ExternalInput")
    # in2p is host-padded: [C, h+8, W+8] with zeros in the 4-wide borders.
    in2p = nc.dram_tensor("in2p", [C, hp, WP], FP32R, kind="ExternalInput")
    out = nc.dram_tensor("out", [K, h, W], FP32, kind="ExternalOutput")
    out_t = out[:, :, :].tensor

    with TileContext(nc) as tc:
        with (
            tc.tile_pool(name="big", bufs=1) as big_pool,
            tc.tile_pool(name="work", bufs=3) as work_pool,
            tc.tile_pool(name="gpsum", bufs=2, space="PSUM") as gpsum,
            tc.tile_pool(name="tpsum", bufs=2, space="PSUM") as tpsum,
        ):
            s1 = big_pool.tile([C, h, W], FP32R, name="s1")
            s2p = big_pool.tile([C, hp, WP], FP32R, name="s2p")
            ident = big_pool.tile([128, 128], FP32, name="ident")
            make_identity(nc, ident)

            # Load inputs in row-chunks so compute can start early.
            nchunk = 4
            rows1 = (h + nchunk - 1) // nchunk
            for i in range(0, h, rows1):
                r = min(rows1, h - i)
                nc.sync.dma_start(s1[:, i : i + r, :], in1[:, i : i + r, :])
            rows2 = (hp + nchunk - 1) // nchunk
            for i in range(0, hp, rows2):
                r = min(rows2, hp - i)
                nc.sync.dma_start(s2p[:, i : i + r, :], in2p[:, i : i + r, :])

            for y in range(h):
                # --- 3 matmuls: G[x, (dy, x')] over dy triplets ---
                gp = gpsum.tile([128, 3, 512], FP32, name="gp", tag="gp")
                for j in range(3):
                    nc.tensor.matmul(
                        gp[:, j, 0 : 3 * WP],
                        s1[:, y, :],
                        s2p[:, y + 3 * j : y + 3 * j + 3, :],
                        start=True,
                        stop=True,
                    )

                # --- PSUM -> SBUF windowed copy (per 32-partition group) ---
                # W[x, dy, u] = G[x, dy, n = 32*(x//32) + u], u in [0, 40).
                # The group base 32g is absorbed into each copy's offsets, so
                # every AP is pure (no partition/byte mixed strides); engine
                # partition bases must be multiples of 32.
                wt = work_pool.tile([128, D, 40], FP32, name="wt", tag="wt")
                # view gp as [p, j, r, n] with n = moving col within dy row
                gp_r = gp[:, :, 0 : 3 * WP].rearrange(
                    "p j (r n) -> p j r n", r=3
                )
                wt_r = wt[:, :, :].rearrange("p (j r) u -> p j r u", j=3)
                for g in range(4):
                    src = gp_r[32 * g : 32 * g + 32, :, :, 32 * g : 32 * g + 40]
                    dst = wt_r[32 * g : 32 * g + 32, :, :, :]
                    if g % 2 == 0:
                        nc.scalar.activation(
                            dst, src, mybir.ActivationFunctionType.Copy
                        )
                    else:
                        nc.vector.tensor_copy(dst, src)

                # --- band extraction: 32 partition-strided SBUF->SBUF DMAs ---
                # For s = x mod 32: t2[x, dy*9+dx] = W[x, dy, s+dx]
                t2 = work_pool.tile([128, K], FP32, name="t2", tag="t2")
                for s in range(32):
                    src = wt[s::32, :, s : s + D]
                    dst = t2[s::32, :]
                    eng = nc.scalar if s % 2 == 0 else nc.sync
                    eng.dma_start(dst, src)

                # --- PE transpose [128, 81] -> [81, 128] ---
                tt = tpsum.tile([K, 128], FP32, name="tt", tag="tt")
                nc.tensor.transpose(tt[:, :], t2[:, :], ident[:, :])

                # --- scale by 1/C and copy to SBUF ---
                to = work_pool.tile([K, 128], FP32, name="to", tag="to")
                nc.scalar.activation(
                    to[:, :],
                    tt[:, :],
                    mybir.ActivationFunctionType.Copy,
                    scale=1.0 / C,
                )

                # --- store: partition k = dy*9+dx -> out[k, y, :] ---
                nc.sync.dma_start(out[:, y, :], to[:, :])

    nc.compile()
    return nc


_cached = {}


def _get_nc(h: int):
    if h not in _cached:
        _cached[h] = build_bass(h)
    return _cached[h]


def _pad_in2(in2: np.ndarray) -> np.ndarray:
    # [C, h, W] -> [C, h+8, W+8] zero-padded, contiguous fp32
    return np.pad(
        in2, ((0, 0), (PAD, PAD), (PAD, PAD)), mode="constant"
    ).astype(np.float32, copy=False)


def kernel(**inputs: np.ndarray) -> np.ndarray:
    in1 = np.ascontiguousarray(inputs["in1"], dtype=np.float32)
    in2 = np.ascontiguousarray(inputs["in2"], dtype=np.float32)
    assert in1.shape == (B, C, H, W), in1.shape

    nc = _get_nc(H)
    in_maps = [
        {
            "in1": np.ascontiguousarray(in1[b]),
            "in2p": np.ascontiguousarray(_pad_in2(in2[b])),
        }
        for b in range(B)
    ]
    res = run_bass_kernel_spmd(nc, in_maps, core_ids=list(range(N_CORES)))
    return np.stack([r["out"] for r in res.results], axis=0)



# revision 10
# speedup vs baseline: 2.2057x; 2.2057x over previous
"""Correlation cost-volume kernel for Trainium2 (Bass/Tile).

Problem: in1, in2: [B=8, C=128, H=96, W=128] fp32.
Output: [B, 81, H, W] where out[b, dy*9+dx, y, x] =
    mean_c( in1[b,c,y,x] * in2_pad[b,c,y+dy,x+dx] ),
with in2 zero-padded by 4 in both spatial dims (max_displacement=4).

Strategy (data-parallel over batch, one sample per NeuronCore), fp16:
  - 4-row blocks: per block and per 32-pixel group g, ONE matmul with
    stationary = in1[:, yb:yb+4, 32g:32g+32] ([C, 4 rows, 32 px] ->
    M=128, PSUM partitions m = 32r+u at base 0, which is the only base
    the MM ISA allows) and moving = s2p[:, yb:yb+12, 32g:32g+40]
    ([C, 12, 40], N=480). The 12 dy' rows cover dy' = r+dy for all
    r in [0,4), dy in [0,9). fp16 runs 1 cycle/row at any N, so
    sharing the moving across 4 rows cuts PE work 3x vs per-row
    matmuls (480 vs 1440 cycles/row).
  - Full-width [128, 960] PSUM->SBUF copies (2 per block, Scalar /
    Vector) into wt4[m, blk, g, dy', v] (fp16).
  - 4 "unshift" DMAs per 32-row chunk remove the r-dependent dy'
    offset (partition base 32r, byte offset r*dy_stride: pure APs):
    wtf[32r+u, blk, g, dy, v] = wt4[32r+u, blk, g, r+dy, v].
  - 32 batched shear DMAs per chunk (one per residue u) extract the
    banded taps: t2f[m, blk, g, dy, dx] = wtf[m, blk, g, dy, u+dx]
    where u = m mod 32. (The baseline issued 32 tiny DMAs per ROW =
    3072 total; each dma_start costs the issuing engine ~0.6-1.3 us,
    which was 90% of its runtime.)
  - PE-transpose t2f[:, blk, g, :] ([128 m, 81 k] -> [81, 128]); the
    r-interleave (m = 32r+u) is undone for free by the drain/output
    access patterns: out[k, yb+r, 32g+u] <- tt[k, g, 32r+u].
  - Scalar/Vector drain PSUM->SBUF with the 1/C scale; one output DMA
    per chunk (contiguous 16 KiB per partition).

fp16 inputs: inputs are unit normals, C=128 products accumulate in
fp32 PSUM; measured end-to-end relative error ~5e-3 vs the 2e-2 gate.
"""

import numpy as np

import concourse.bass as bass
import concourse.mybir as mybir
from concourse import bacc
from concourse.bass_utils import run_bass_kernel_spmd
from concourse.masks import make_identity
from concourse.tile import TileContext

B = 8
C = 128
H = 96
W = 128
D = 9  # 2*max_disp + 1
K = D * D  # 81 output channels
PAD = 4
WP = W + 2 * PAD  # 136
FP32 = mybir.dt.float32
FP16 = mybir.dt.float16

N_CORES = 8
RCH = 32  # rows per shear chunk
NBLK = RCH // 4  # 4-row blocks per chunk
COPY = mybir.ActivationFunctionType.Copy


def build_bass(h: int = H):
    """Build the per-core Bass program for a [C, h, W] sample."""
    hp = h + 2 * PAD
    nch = h // RCH
    assert h % RCH == 0
    nc = bacc.Bacc(None, target_bir_lowering=False)
    # in1s is host-shuffled to block-major [C, blk, g, r, u] with
    # y = 4*blk + r, x = 32*g + u, so each matmul's stationary
    # ([C, 128] = 4 rows x 32 px of one group) is one contiguous dim
    # (the MM ISA allows only one free dim on the weights AP).
    in1s = nc.dram_tensor(
        "in1s", [C, h // 4, 4, 4, 32], FP16, kind="ExternalInput"
    )
    # in2p is host-padded: [C, h+8, W+8] with zeros in the 4-wide borders.
    in2p = nc.dram_tensor("in2p", [C, hp, WP], FP16, kind="ExternalInput")
    out = nc.dram_tensor("out", [K, h, W], FP32, kind="ExternalOutput")

    with TileContext(nc) as tc:
        with (
            tc.tile_pool(name="cst", bufs=1) as cst,
            tc.tile_pool(name="s1p", bufs=2) as s1p,
            tc.tile_pool(name="wt4p", bufs=2) as wt4p,
            tc.tile_pool(name="wtfp", bufs=1) as wtfp,
            tc.tile_pool(name="t2p", bufs=2) as t2p,
            tc.tile_pool(name="top", bufs=2) as top,
            tc.tile_pool(name="gpp", bufs=3, space="PSUM") as gpp,
            tc.tile_pool(name="ttp", bufs=2, space="PSUM") as ttp,
        ):
            s2p = cst.tile([C, hp, WP], FP16, name="s2p")
            ident = cst.tile([128, 128], FP16, name="ident")
            make_identity(nc, ident)

            # Load the padded in2 plane in row chunks so compute starts early.
            n2 = 4
            rows2 = (hp + n2 - 1) // n2
            for i in range(0, hp, rows2):
                r = min(rows2, hp - i)
                nc.sync.dma_start(s2p[:, i : i + r, :], in2p[:, i : i + r, :])

            for ch in range(nch):
                y0 = ch * RCH
                # s1c[c, blk, g, m] with m = 32r+u (one contiguous 128 dim)
                s1c = s1p.tile([C, NBLK, 4, 128], FP16, name="s1c", tag="s1c")
                b0 = ch * NBLK
                nc.sync.dma_start(
                    s1c[:, :, :, :].rearrange("c b g (r u) -> c b g r u", r=4),
                    in1s[:, b0 : b0 + NBLK, :, :, :],
                )

                # wt4[m=32r+u, blk, g, dy', v]
                wt4 = wt4p.tile([128, NBLK, 4, 12, 40], FP16, name="wt4", tag="wt4")
                for blk in range(NBLK):
                    yb = y0 + 4 * blk
                    # two 2-bank PSUM tiles per block: g in {0,1} / {2,3}
                    for half in range(2):
                        gp = gpp.tile([128, 2, 512], FP32, name="gp", tag="gp")
                        for j in range(2):
                            g = 2 * half + j
                            nc.tensor.matmul(
                                gp[:, j, 0:480].rearrange(
                                    "p (dy v) -> p dy v", dy=12
                                ),
                                s1c[:, blk, g, :],
                                s2p[:, yb : yb + 12, 32 * g : 32 * g + 40],
                                start=True,
                                stop=True,
                            )
                        # full-width windowed PSUM -> SBUF copy (fp32 -> fp16)
                        src = gp[:, :, 0:480].rearrange(
                            "p j (dy v) -> p j dy v", dy=12
                        )
                        dst = wt4[:, blk, 2 * half : 2 * half + 2, :, :]
                        if half == 0:
                            nc.scalar.activation(dst, src, COPY)
                        else:
                            nc.vector.tensor_copy(dst, src)

                # --- dy-unshift: 4 partition-block DMAs ---
                # wtf[32r+u, blk, g, dy, v] = wt4[32r+u, blk, g, r+dy, v]
                wtf = wtfp.tile([128, NBLK, 4, D, 40], FP16, name="wtf", tag="wtf")
                for r in range(4):
                    eng = (nc.sync, nc.gpsimd, nc.sync, nc.gpsimd)[r]
                    eng.dma_start(
                        wtf[32 * r : 32 * r + 32, :, :, :, :],
                        wt4[32 * r : 32 * r + 32, :, :, r : r + D, :],
                    )

                # --- batched band extraction: 32 partition-strided DMAs ---
                # For u = m mod 32: t2f[m, blk, g, dy, dx] = wtf[m, blk, g, dy, u+dx]
                t2f = t2p.tile([128, NBLK, 4, D, D], FP16, name="t2f", tag="t2f")
                for s in range(32):
                    src = wtf[s::32, :, :, :, s : s + D]
                    dst = t2f[s::32, :, :, :, :]
                    eng = (nc.sync, nc.scalar, nc.gpsimd, nc.sync)[s % 4]
                    eng.dma_start(dst, src)

                # --- PE transpose [128 m, 81 k] -> [81, 128 m], drain, store ---
                to = top.tile([K, RCH, W], FP32, name="to", tag="to")
                for blk in range(NBLK):
                    tt = ttp.tile([K, 4, 128], FP16, name="tt", tag="tt")
                    for g in range(4):
                        nc.tensor.transpose(
                            tt[:, g, :], t2f[:, blk, g, :, :], ident[:, :]
                        )
                    # drain + 1/C scale; undo the m = 32r+u interleave:
                    # to[k, 4blk+r, 32g+u] <- tt[k, g, 32r+u]
                    dst = to[:, 4 * blk : 4 * blk + 4, :].rearrange(
                        "k r (g u) -> k r g u", g=4
                    )
                    src = tt[:, :, :].rearrange("k g (r u) -> k r g u", r=4)
                    if blk % 2 == 0:
                        nc.scalar.activation(dst, src, COPY, scale=1.0 / C)
                    else:
                        nc.vector.tensor_scalar_mul(dst, src, 1.0 / C)

                # --- store: contiguous [81, RCH*W] block ---
                nc.sync.dma_start(out[:, y0 : y0 + RCH, :], to[:, :, :])

    nc.compile()
    return nc


_cached = {}


def _get_nc(h: int):
    if h not in _cached:
        _cached[h] = build_bass(h)
    return _cached[h]


def _pad_in2(in2: np.ndarray) -> np.ndarray:
    # [C, h, W] fp16 -> [C, h+8, W+8] zero-padded, contiguous fp16
    return np.pad(
        in2.astype(np.float16), ((0, 0), (PAD, PAD), (PAD, PAD)), mode="constant"
    )


def _shuffle_in1(in1: np.ndarray) -> np.ndarray:
    # [C, h, W] -> [C, h//4, 4(g), 4(r), 32(u)]: block-major stationary
    # layout with y = 4*blk + r, x = 32*g + u.
    c, h, w = in1.shape
    a = in1.astype(np.float16).reshape(c, h // 4, 4, 4, 32)  # c, blk, r, g, u
    return np.ascontiguousarray(a.transpose(0, 1, 3, 2, 4))  # c, blk, g, r, u


def kernel(**inputs: np.ndarray) -> np.ndarray:
    in1 = np.asarray(inputs["in1"], dtype=np.float32)
    in2 = np.asarray(inputs["in2"], dtype=np.float32)
    assert in1.shape == (B, C, H, W), in1.shape

    nc = _get_nc(H)
    in_maps = [
        {
            "in1s": _shuffle_in1(in1[b]),
            "in2p": np.ascontiguousarray(_pad_in2(in2[b])),
        }
        for b in range(B)
    ]
    res = run_bass_kernel_spmd(nc, in_maps, core_ids=list(range(N_CORES)))
    return np.stack([r["out"] for r in res.results], axis=0)
